# revision 16
# baseline (speedup 1.0000x reference)
"""BitLinear (RMSNorm + per-tensor int8 act-quant + ternary weight matmul) on 8 NeuronCores.

Sharding: data-parallel over tokens (B*S=16384 -> 2048/core); weight replicated.
Global activation scale (max|xn|) via AllReduce(max); weight scale (mean|w|) via
per-core shard scan + AllReduce(add).

Math notes:
 - matmul runs in bf16 on integer-valued operands (acts in [-127,127], weights
   ternary) with fp32 PSUM accumulation => bit-exact integer dot products.
 - round-half-to-even is implemented with the fp32 magic constant 1.5*2^23:
   ACT's activation op computes fma(x, scale, MAGIC) (single rounding), which
   rounds x*scale to the nearest integer (RNE), then we subtract MAGIC.

Ring assignment: sync HWDGE ring: x pass-1 loads, weight loads, all xbar
transposes. scalar HWDGE ring: x pass-2 loads, scale fetch, output stores.
gpsimd SWDGE: weight-shard loads, collective bounces.
"""

import sys

import numpy as np

sys.path.insert(0, "/opt/trn_rl_repo")
sys.path.insert(0, "/opt/trn_rl_repo/concourse")

import concourse.bass as bass  # noqa: E402
import concourse.tile as tile  # noqa: E402
from concourse import bacc, bass_isa, mybir  # noqa: E402
from concourse.bass_utils import run_bass_kernel_spmd  # noqa: E402


def enable_profiling():
    """Register the axon NTFF profile hook (the image lacks antenv.axon_hooks)."""
    import types
    if "antenv.axon_hooks" in sys.modules:
        return
    try:
        sys.path.insert(0, "/root/.axon_site")
        from trn_agent_boot.trn_boot import _ntff_profile_via_ctypes
        hook = _ntff_profile_via_ctypes("/opt/axon/libaxon_pjrt.so")
        mod = types.ModuleType("antenv.axon_hooks")
        mod._hook = hook
        mod.get_axon_ntff_profile_hook = lambda: mod._hook
        def _set(h):
            mod._hook = h
        mod.set_axon_ntff_profile_hook = _set
        sys.modules["antenv.axon_hooks"] = mod
    except Exception as e:  # profiling is best-effort
        print(f"enable_profiling failed: {e}")


N_CORES = 8
P = 128
MAGIC = 12582912.0  # 1.5 * 2^23
QP = 127.0
EPS_NORM = 1e-6
EPS_SCALE = 1e-5
F32 = mybir.dt.float32
BF16 = mybir.dt.bfloat16
AF = mybir.ActivationFunctionType
ALU = mybir.AluOpType


def build(T, D, O, OSH):
    """Per-core program. x shard [T, D], weight [O, D], wsh [OSH, D] -> out [T, O]."""
    NT, NI, NW = T // P, D // P, O // P
    NOB = O // 512
    NSH = OSH // P
    nc = bacc.Bacc("TRN2", target_bir_lowering=False, debug=False, num_devices=N_CORES)

    x_in = nc.dram_tensor("x", [T, D], F32, kind="ExternalInput")
    w_in = nc.dram_tensor("w", [O, D], F32, kind="ExternalInput")
    wsh_in = nc.dram_tensor("wsh", [OSH, D], F32, kind="ExternalInput")
    out_t = nc.dram_tensor("out", [T, O], F32, kind="ExternalOutput")

    ccs_in = nc.dram_tensor("ccs_in", [P, 1], F32)
    ccs_out = nc.dram_tensor("ccs_out", [P, 1], F32, addr_space="Shared")
    ccm_in = nc.dram_tensor("ccm_in", [P, 1], F32)
    ccm_out = nc.dram_tensor("ccm_out", [P, 1], F32, addr_space="Shared")
    groups = [list(range(N_CORES))]

    with tile.TileContext(nc) as tc:
        with (
            tc.tile_pool(name="xload", bufs=3) as xload,
            tc.tile_pool(name="wload", bufs=3) as wload,
            tc.tile_pool(name="scr32", bufs=3) as scr32,
            tc.tile_pool(name="scr16", bufs=3) as scr16,
            tc.tile_pool(name="wscr16", bufs=3) as wscr16,
            tc.tile_pool(name="qT", bufs=NT) as qTp,
            tc.tile_pool(name="wqT", bufs=min(2, NOB)) as wqTp,
            tc.tile_pool(name="stats", bufs=1) as stats,
            tc.tile_pool(name="outp", bufs=3) as outp,
            tc.tile_pool(name="psum", bufs=4, space="PSUM") as psump,
        ):
            # ---- const tiles ----
            magic_t = stats.tile([P, 1], F32)
            nc.vector.memset(magic_t[:], MAGIC)
            epsn_t = stats.tile([P, 1], F32)
            nc.vector.memset(epsn_t[:], EPS_NORM)

            # ---- weight-shard abs scan (for global mean|w|) ----
            wabs = stats.tile([P, NSH], F32)
            for s in range(NSH):
                wt = wload.tile([P, D], F32, tag="wload")
                nc.gpsimd.dma_start(out=wt[:], in_=wsh_in.ap()[s * P:(s + 1) * P, :])
                scr = scr16.tile([P, D], BF16, tag="scr16")
                nc.scalar.activation(out=scr[:], in_=wt[:], func=AF.Abs,
                                     accum_out=wabs[:, s:s + 1])
            wsum = stats.tile([P, 1], F32)
            nc.vector.tensor_reduce(out=wsum[:], in_=wabs[:], axis=mybir.AxisListType.X,
                                    op=ALU.add)
            wsumr = stats.tile([P, 1], F32)
            nc.gpsimd.partition_all_reduce(wsumr[:], wsum[:], channels=P,
                                           reduce_op=bass_isa.ReduceOp.add)
            nc.gpsimd.dma_start(out=ccs_in.ap(), in_=wsumr[:])
            nc.gpsimd.collective_compute(
                "AllReduce", ALU.add, replica_groups=groups,
                ins=[ccs_in.ap()], outs=[ccs_out.ap()])
            wsg = stats.tile([P, 1], F32)
            nc.gpsimd.dma_start(out=wsg[:], in_=ccs_out.ap())
            ws = stats.tile([P, 1], F32)
            nc.vector.tensor_scalar(out=ws[:], in0=wsg[:], scalar1=1.0 / (O * D),
                                    scalar2=EPS_SCALE, op0=ALU.mult, op1=ALU.max)
            inv_ws = stats.tile([P, 1], F32)
            nc.vector.reciprocal(out=inv_ws[:], in_=ws[:])

            # ---- pass 1 over x: sumsq (ACT) + absmax (DVE) ----
            sumsq = stats.tile([P, NT], F32)
            rowmax = stats.tile([P, NT], F32)
            x1_lds = []
            for k in range(NT):
                xt = xload.tile([P, D], F32, tag="xload")
                ld = nc.sync.dma_start(out=xt[:], in_=x_in.ap()[k * P:(k + 1) * P, :])
                x1_lds.append(ld)
                scr = scr16.tile([P, D], BF16, tag="scr16")
                nc.scalar.activation(out=scr[:], in_=xt[:], func=AF.Square,
                                     accum_out=sumsq[:, k:k + 1])
                nc.vector.tensor_reduce(out=rowmax[:, k:k + 1], in_=xt[:],
                                        axis=mybir.AxisListType.X, op=ALU.max,
                                        apply_absolute_value=True)

            # ---- stats finalize: rr ~= rsqrt(mean(x^2)+eps), local max ----
            m = stats.tile([P, NT], F32)
            nc.vector.tensor_scalar(out=m[:], in0=sumsq[:], scalar1=1.0 / D,
                                    scalar2=EPS_NORM, op0=ALU.mult, op1=ALU.add)
            rms = stats.tile([P, NT], F32)
            nc.scalar.activation(out=rms[:], in_=sumsq[:], func=AF.Sqrt,
                                 scale=1.0 / D, bias=epsn_t[:])
            rr = stats.tile([P, NT], F32)
            nc.vector.reciprocal(out=rr[:], in_=rms[:])
            # 2 Newton steps for rsqrt(m): r <- r*(1.5 - 0.5*m*r*r)
            for it in range(2):
                t1 = stats.tile([P, NT], F32, tag=f"nwt1_{it}", name=f"nwt1_{it}")
                nc.vector.tensor_tensor(out=t1[:], in0=rr[:], in1=rr[:], op=ALU.mult)
                t2 = stats.tile([P, NT], F32, tag=f"nwt2_{it}", name=f"nwt2_{it}")
                nc.vector.tensor_tensor(out=t2[:], in0=t1[:], in1=m[:], op=ALU.mult)
                t3 = stats.tile([P, NT], F32, tag=f"nwt3_{it}", name=f"nwt3_{it}")
                nc.vector.tensor_scalar(out=t3[:], in0=t2[:], scalar1=-0.5,
                                        scalar2=1.5, op0=ALU.mult, op1=ALU.add)
                rr2 = stats.tile([P, NT], F32, tag=f"nwt4_{it}", name=f"nwt4_{it}")
                nc.vector.tensor_tensor(out=rr2[:], in0=rr[:], in1=t3[:], op=ALU.mult)
                rr = rr2

            tm = stats.tile([P, NT], F32)
            nc.vector.tensor_tensor(out=tm[:], in0=rowmax[:], in1=rr[:], op=ALU.mult)
            lm = stats.tile([P, 1], F32)
            nc.vector.tensor_reduce(out=lm[:], in_=tm[:], axis=mybir.AxisListType.X,
                                    op=ALU.max)
            pm = stats.tile([P, 1], F32)
            nc.gpsimd.partition_all_reduce(pm[:], lm[:], channels=P,
                                           reduce_op=bass_isa.ReduceOp.max)
            nc.gpsimd.dma_start(out=ccm_in.ap(), in_=pm[:])
            cc2 = nc.gpsimd.collective_compute(
                "AllReduce", ALU.max, replica_groups=groups,
                ins=[ccm_in.ap()], outs=[ccm_out.ap()])

            from concourse.tile_rust import add_dep_helper

            # ---- fetch global max, build quant scales ----
            gmb = stats.tile([P, 1], F32)
            nc.scalar.dma_start(out=gmb[:], in_=ccm_out.ap())
            scale_g = stats.tile([P, 1], F32)
            nc.vector.tensor_scalar(out=scale_g[:], in0=gmb[:], scalar1=EPS_SCALE,
                                    scalar2=None, op0=ALU.max)
            cr0 = stats.tile([P, 1], F32)
            nc.vector.reciprocal(out=cr0[:], in_=scale_g[:])
            cq = stats.tile([P, 1], F32)
            nc.vector.tensor_scalar(out=cq[:], in0=cr0[:], scalar1=QP, scalar2=None,
                                    op0=ALU.mult)
            cfull = stats.tile([P, NT], F32)
            nc.vector.tensor_scalar(out=cfull[:], in0=rr[:], scalar1=cq[:],
                                    scalar2=None, op0=ALU.mult)
            # final output scale = scale_g * ws / 127
            fs0 = stats.tile([P, 1], F32)
            nc.vector.tensor_tensor(out=fs0[:], in0=scale_g[:], in1=ws[:], op=ALU.mult)
            fs = stats.tile([P, 1], F32)
            nc.vector.tensor_scalar(out=fs[:], in0=fs0[:], scalar1=1.0 / QP,
                                    scalar2=None, op0=ALU.mult)

            # ---- weight quantize helper (load -> round/clip -> transpose) ----
            per_ob = NW // NOB
            wqT = {}

            def weight_chunk(wk, gate_ld, wqT_tile):
                wt = wload.tile([P, D], F32, tag="wload", name=f"wt{wk}")
                ld = nc.sync.dma_start(out=wt[:],
                                       in_=w_in.ap()[wk * P:(wk + 1) * P, :])
                add_dep_helper(ld.ins, gate_ld.ins, True, "HBM ordering")
                yw = scr32.tile([P, D], F32, tag="scr32", name=f"yw{wk}")
                nc.scalar.activation(out=yw[:], in_=wt[:], func=AF.Identity,
                                     scale=inv_ws[:], bias=magic_t[:])
                zw = wscr16.tile([P, D], BF16, tag="wscr16", name=f"zw{wk}")
                nc.vector.tensor_scalar(out=zw[:], in0=yw[:], scalar1=MAGIC,
                                        scalar2=1.0, op0=ALU.subtract, op1=ALU.min)
                wqn = wscr16.tile([P, D], BF16, tag="wscr16", name=f"wqn{wk}")
                nc.vector.tensor_scalar(out=wqn[:], in0=zw[:], scalar1=-1.0,
                                        scalar2=None, op0=ALU.max)
                col = wk % per_ob
                nc.scalar.dma_start_transpose(
                    out=wqT_tile[:, :, col * P:(col + 1) * P], in_=wqn[:])
                return ld

            # first o-block's weights right after x1 on HBM
            wqT[0] = wqTp.tile([P, NI, 512], BF16, tag="wqT", name="wqT0")
            gate = x1_lds[-1]
            for wk in range(per_ob):
                gate = weight_chunk(wk, gate, wqT[0])

            # ---- x pass 2 loads: after first weight block on HBM ----
            xt2s = []
            x2_lds = []
            for k in range(NT):
                xt2 = xload.tile([P, D], F32, tag="xload", name=f"xt2_{k}")
                ld = nc.scalar.dma_start(out=xt2[:],
                                         in_=x_in.ap()[k * P:(k + 1) * P, :])
                add_dep_helper(ld.ins, gate.ins, True, "x2 after wq0 on HBM")
                xt2s.append(xt2)
                x2_lds.append(ld)

            # ---- x pass 2 quantize + transpose ----
            qTs = []
            for k in range(NT):
                yq = scr32.tile([P, D], F32, tag="scr32", name=f"yq{k}")
                nc.scalar.activation(out=yq[:], in_=xt2s[k][:], func=AF.Identity,
                                     scale=cfull[:, k:k + 1], bias=magic_t[:])
                qn = scr16.tile([P, D], BF16, tag="scr16", name=f"qn{k}")
                nc.vector.tensor_scalar(out=qn[:], in0=yq[:], scalar1=MAGIC,
                                        scalar2=None, op0=ALU.subtract)
                qT = qTp.tile([P, NI, P], BF16, tag="qT", name=f"qT{k}")
                nc.sync.dma_start_transpose(out=qT[:], in_=qn[:])
                qTs.append(qT)

            # ---- remaining weight blocks: after x2 on HBM ----
            gate = x2_lds[-1]
            for ob in range(1, NOB):
                wqT[ob] = wqTp.tile([P, NI, 512], BF16, tag="wqT", name=f"wqT{ob}")
                for wk in range(ob * per_ob, (ob + 1) * per_ob):
                    gate = weight_chunk(wk, gate, wqT[ob])

            # ---- matmul (ob-outer): psum[t,o] += qT_k[:,i,:].T @ wqT_ob[:,i,:] ----
            for ob in range(NOB):
                for k in range(NT):
                    ps = psump.tile([P, 512], F32, tag="ps", name=f"ps{ob}_{k}")
                    for i in range(NI):
                        nc.tensor.matmul(ps[:], qTs[k][:, i, :], wqT[ob][:, i, :],
                                         start=(i == 0), stop=(i == NI - 1))
                    osb = outp.tile([P, 512], F32, tag="osb", name=f"osb{ob}_{k}")
                    if (k + ob) % 2 == 0:
                        nc.vector.tensor_scalar(out=osb[:], in0=ps[:],
                                                scalar1=fs[:], scalar2=None,
                                                op0=ALU.mult)
                    else:
                        nc.scalar.activation(out=osb[:], in_=ps[:],
                                             func=AF.Identity, scale=fs[:])
                    nc.scalar.dma_start(
                        out=out_t.ap()[k * P:(k + 1) * P, ob * 512:(ob + 1) * 512],
                        in_=osb[:])

    nc.compile()
    return nc


_cache = {}


def _get(T, D, O, OSH):
    key = (T, D, O, OSH)
    if key not in _cache:
        _cache[key] = build(T, D, O, OSH)
    return _cache[key]


def run(x2d, weight, osh=None, trace=False):
    """x2d: [Ttot, D] f32, weight: [O, D] f32 -> [Ttot, O] f32."""
    Ttot, D = x2d.shape
    O = weight.shape[0]
    T = Ttot // N_CORES
    OSH = osh or O // N_CORES
    nc = _get(T, D, O, OSH)
    in_maps = []
    for c in range(N_CORES):
        in_maps.append({
            "x": np.ascontiguousarray(x2d[c * T:(c + 1) * T]),
            "w": weight,
            "wsh": np.ascontiguousarray(weight[c * OSH:(c + 1) * OSH]),
        })
    res = run_bass_kernel_spmd(nc, in_maps, core_ids=list(range(N_CORES)),
                               trace=trace)
    out = np.concatenate([res.results[c]["out"] for c in range(N_CORES)], axis=0)
    return out, res


def kernel(x, weight, norm_weight):
    assert np.all(norm_weight == 1.0), "general norm_weight not implemented"
    B, S, D = x.shape
    out2d, _ = run(x.reshape(B * S, D).astype(np.float32), weight.astype(np.float32))
    return out2d.reshape(B, S, weight.shape[0]).astype(np.float32)


# revision 17
# speedup vs baseline: 1.1307x; 1.1307x over previous
"""BitLinear (RMSNorm + per-tensor int8 act-quant + ternary weight matmul) on 8 NeuronCores.

Sharding: data-parallel over tokens (B*S=16384 -> 2048/core); weight replicated.
Global activation scale (max|xn|) via AllReduce(max); weight scale (mean|w|) via
per-core shard scan + AllReduce(add).

Math notes:
 - matmul runs in bf16 on integer-valued operands (acts in [-127,127], weights
   ternary) with fp32 PSUM accumulation => bit-exact integer dot products.
 - round-half-to-even is implemented with the fp32 magic constant 1.5*2^23:
   ACT's activation op computes fma(x, scale, MAGIC) (single rounding), which
   rounds x*scale to the nearest integer (RNE), then we subtract MAGIC.

Ring assignment: sync HWDGE ring: x pass-1 loads, weight loads, all xbar
transposes. scalar HWDGE ring: x pass-2 loads, scale fetch, output stores.
gpsimd SWDGE: weight-shard loads, collective bounces.
"""

import sys

import numpy as np

sys.path.insert(0, "/opt/trn_rl_repo")
sys.path.insert(0, "/opt/trn_rl_repo/concourse")

import concourse.bass as bass  # noqa: E402
import concourse.tile as tile  # noqa: E402
from concourse import bacc, bass_isa, mybir  # noqa: E402
from concourse.bass_utils import run_bass_kernel_spmd  # noqa: E402


def enable_profiling():
    """Register the axon NTFF profile hook (the image lacks antenv.axon_hooks)."""
    import types
    if "antenv.axon_hooks" in sys.modules:
        return
    try:
        sys.path.insert(0, "/root/.axon_site")
        from trn_agent_boot.trn_boot import _ntff_profile_via_ctypes
        hook = _ntff_profile_via_ctypes("/opt/axon/libaxon_pjrt.so")
        mod = types.ModuleType("antenv.axon_hooks")
        mod._hook = hook
        mod.get_axon_ntff_profile_hook = lambda: mod._hook
        def _set(h):
            mod._hook = h
        mod.set_axon_ntff_profile_hook = _set
        sys.modules["antenv.axon_hooks"] = mod
    except Exception as e:  # profiling is best-effort
        print(f"enable_profiling failed: {e}")


N_CORES = 8
P = 128
MAGIC = 12582912.0  # 1.5 * 2^23
QP = 127.0
EPS_NORM = 1e-6
EPS_SCALE = 1e-5
F32 = mybir.dt.float32
BF16 = mybir.dt.bfloat16
AF = mybir.ActivationFunctionType
ALU = mybir.AluOpType


def build(T, D, O, OSH):
    """Per-core program. x shard [T, D], weight [O, D], wsh [OSH, D] -> out [T, O]."""
    NT, NI, NW = T // P, D // P, O // P
    NOB = O // 512
    NSH = OSH // P
    nc = bacc.Bacc("TRN2", target_bir_lowering=False, debug=False, num_devices=N_CORES)

    x_in = nc.dram_tensor("x", [T, D], F32, kind="ExternalInput")
    w_in = nc.dram_tensor("w", [O, D], F32, kind="ExternalInput")
    wsh_in = nc.dram_tensor("wsh", [OSH, D], F32, kind="ExternalInput")
    out_t = nc.dram_tensor("out", [T, O], F32, kind="ExternalOutput")

    ccs_in = nc.dram_tensor("ccs_in", [P, 1], F32)
    ccs_out = nc.dram_tensor("ccs_out", [P, 1], F32, addr_space="Shared")
    ccm_in = nc.dram_tensor("ccm_in", [P, 1], F32)
    ccm_out = nc.dram_tensor("ccm_out", [P, 1], F32, addr_space="Shared")
    groups = [list(range(N_CORES))]

    with tile.TileContext(nc) as tc:
        with (
            tc.tile_pool(name="xload", bufs=3) as xload,
            tc.tile_pool(name="wload", bufs=2) as wload,
            tc.tile_pool(name="scr32", bufs=3) as scr32,
            tc.tile_pool(name="scr16", bufs=4) as scr16,
            tc.tile_pool(name="wscr16", bufs=5) as wscr16,
            tc.tile_pool(name="qT", bufs=NT) as qTp,
            tc.tile_pool(name="wqT", bufs=min(2, NOB)) as wqTp,
            tc.tile_pool(name="stats", bufs=1) as stats,
            tc.tile_pool(name="outp", bufs=2) as outp,
            tc.tile_pool(name="psum", bufs=4, space="PSUM") as psump,
        ):
            # ---- const tiles ----
            magic_t = stats.tile([P, 1], F32)
            nc.vector.memset(magic_t[:], MAGIC)
            epsn_t = stats.tile([P, 1], F32)
            nc.vector.memset(epsn_t[:], EPS_NORM)

            # ---- weight-shard abs scan (for global mean|w|) ----
            wabs = stats.tile([P, NSH], F32)
            for s in range(NSH):
                wt = wload.tile([P, D], F32, tag="wload")
                nc.gpsimd.dma_start(out=wt[:], in_=wsh_in.ap()[s * P:(s + 1) * P, :])
                scr = scr16.tile([P, D], BF16, tag="sqscr", bufs=1)
                nc.scalar.activation(out=scr[:], in_=wt[:], func=AF.Abs,
                                     accum_out=wabs[:, s:s + 1])
            wsum = stats.tile([P, 1], F32)
            nc.vector.tensor_reduce(out=wsum[:], in_=wabs[:], axis=mybir.AxisListType.X,
                                    op=ALU.add)
            wsumr = stats.tile([P, 1], F32)
            nc.gpsimd.partition_all_reduce(wsumr[:], wsum[:], channels=P,
                                           reduce_op=bass_isa.ReduceOp.add)
            nc.gpsimd.dma_start(out=ccs_in.ap(), in_=wsumr[:])
            nc.gpsimd.collective_compute(
                "AllReduce", ALU.add, replica_groups=groups,
                ins=[ccs_in.ap()], outs=[ccs_out.ap()])
            wsg = stats.tile([P, 1], F32)
            nc.gpsimd.dma_start(out=wsg[:], in_=ccs_out.ap())
            ws = stats.tile([P, 1], F32)
            nc.vector.tensor_scalar(out=ws[:], in0=wsg[:], scalar1=1.0 / (O * D),
                                    scalar2=EPS_SCALE, op0=ALU.mult, op1=ALU.max)
            inv_ws = stats.tile([P, 1], F32)
            nc.vector.reciprocal(out=inv_ws[:], in_=ws[:])

            # ---- pass 1 over x: sumsq (ACT) + absmax (DVE) ----
            sumsq = stats.tile([P, NT], F32)
            rowmax = stats.tile([P, NT], F32)
            x1_lds = []
            for k in range(NT):
                xt = xload.tile([P, D], F32, tag="xload")
                ld = nc.sync.dma_start(out=xt[:], in_=x_in.ap()[k * P:(k + 1) * P, :])
                x1_lds.append(ld)
                scr = scr16.tile([P, D], BF16, tag="sqscr", bufs=1)
                nc.scalar.activation(out=scr[:], in_=xt[:], func=AF.Square,
                                     accum_out=sumsq[:, k:k + 1])
                nc.vector.tensor_reduce(out=rowmax[:, k:k + 1], in_=xt[:],
                                        axis=mybir.AxisListType.X, op=ALU.max,
                                        apply_absolute_value=True)

            # ---- stats finalize: rr ~= rsqrt(mean(x^2)+eps), local max ----
            m = stats.tile([P, NT], F32)
            nc.vector.tensor_scalar(out=m[:], in0=sumsq[:], scalar1=1.0 / D,
                                    scalar2=EPS_NORM, op0=ALU.mult, op1=ALU.add)
            rms = stats.tile([P, NT], F32)
            nc.scalar.activation(out=rms[:], in_=sumsq[:], func=AF.Sqrt,
                                 scale=1.0 / D, bias=epsn_t[:])
            rr = stats.tile([P, NT], F32)
            nc.vector.reciprocal(out=rr[:], in_=rms[:])
            # 2 Newton steps for rsqrt(m): r <- r*(1.5 - 0.5*m*r*r)
            for it in range(2):
                t1 = stats.tile([P, NT], F32, tag=f"nwt1_{it}", name=f"nwt1_{it}")
                nc.vector.tensor_tensor(out=t1[:], in0=rr[:], in1=rr[:], op=ALU.mult)
                t2 = stats.tile([P, NT], F32, tag=f"nwt2_{it}", name=f"nwt2_{it}")
                nc.vector.tensor_tensor(out=t2[:], in0=t1[:], in1=m[:], op=ALU.mult)
                t3 = stats.tile([P, NT], F32, tag=f"nwt3_{it}", name=f"nwt3_{it}")
                nc.vector.tensor_scalar(out=t3[:], in0=t2[:], scalar1=-0.5,
                                        scalar2=1.5, op0=ALU.mult, op1=ALU.add)
                rr2 = stats.tile([P, NT], F32, tag=f"nwt4_{it}", name=f"nwt4_{it}")
                nc.vector.tensor_tensor(out=rr2[:], in0=rr[:], in1=t3[:], op=ALU.mult)
                rr = rr2

            tm = stats.tile([P, NT], F32)
            nc.vector.tensor_tensor(out=tm[:], in0=rowmax[:], in1=rr[:], op=ALU.mult)
            lm = stats.tile([P, 1], F32)
            nc.vector.tensor_reduce(out=lm[:], in_=tm[:], axis=mybir.AxisListType.X,
                                    op=ALU.max)
            pm = stats.tile([P, 1], F32)
            nc.gpsimd.partition_all_reduce(pm[:], lm[:], channels=P,
                                           reduce_op=bass_isa.ReduceOp.max)
            nc.gpsimd.dma_start(out=ccm_in.ap(), in_=pm[:])
            cc2 = nc.gpsimd.collective_compute(
                "AllReduce", ALU.max, replica_groups=groups,
                ins=[ccm_in.ap()], outs=[ccm_out.ap()])

            from concourse.tile_rust import add_dep_helper

            # ---- fetch global max, build quant scales ----
            gmb = stats.tile([P, 1], F32)
            nc.scalar.dma_start(out=gmb[:], in_=ccm_out.ap())
            scale_g = stats.tile([P, 1], F32)
            nc.vector.tensor_scalar(out=scale_g[:], in0=gmb[:], scalar1=EPS_SCALE,
                                    scalar2=None, op0=ALU.max)
            cr0 = stats.tile([P, 1], F32)
            nc.vector.reciprocal(out=cr0[:], in_=scale_g[:])
            cq = stats.tile([P, 1], F32)
            nc.vector.tensor_scalar(out=cq[:], in0=cr0[:], scalar1=QP, scalar2=None,
                                    op0=ALU.mult)
            cfull = stats.tile([P, NT], F32)
            nc.vector.tensor_scalar(out=cfull[:], in0=rr[:], scalar1=cq[:],
                                    scalar2=None, op0=ALU.mult)
            # final output scale = scale_g * ws / 127
            fs0 = stats.tile([P, 1], F32)
            nc.vector.tensor_tensor(out=fs0[:], in0=scale_g[:], in1=ws[:], op=ALU.mult)
            fs = stats.tile([P, 1], F32)
            nc.vector.tensor_scalar(out=fs[:], in0=fs0[:], scalar1=1.0 / QP,
                                    scalar2=None, op0=ALU.mult)

            # ---- weight quantize helper (load -> round/clip -> transpose) ----
            per_ob = NW // NOB
            wqT = {}

            def weight_chunk(wk, gate_ld, wqT_tile):
                wt = wload.tile([P, D], F32, tag="wload", name=f"wt{wk}")
                ld = nc.sync.dma_start(out=wt[:],
                                       in_=w_in.ap()[wk * P:(wk + 1) * P, :])
                yw = scr32.tile([P, D], F32, tag="scr32", name=f"yw{wk}")
                nc.scalar.activation(out=yw[:], in_=wt[:], func=AF.Identity,
                                     scale=inv_ws[:], bias=magic_t[:])
                zw = wscr16.tile([P, D], BF16, tag="zw", bufs=2, name=f"zw{wk}")
                nc.vector.tensor_scalar(out=zw[:], in0=yw[:], scalar1=MAGIC,
                                        scalar2=1.0, op0=ALU.subtract, op1=ALU.min)
                wqn = wscr16.tile([P, D], BF16, tag="wqn", bufs=3, name=f"wqn{wk}")
                nc.vector.tensor_scalar(out=wqn[:], in0=zw[:], scalar1=-1.0,
                                        scalar2=None, op0=ALU.max)
                col = wk % per_ob
                nc.scalar.dma_start_transpose(
                    out=wqT_tile[:, :, col * P:(col + 1) * P], in_=wqn[:])
                return ld

            # first o-block's weights right after x1 on HBM
            wqT[0] = wqTp.tile([P, NI, 512], BF16, tag="wqT", name="wqT0")
            gate = x1_lds[-1]
            for wk in range(per_ob):
                gate = weight_chunk(wk, gate, wqT[0])

            # ---- x pass 2 loads: after first weight block on HBM ----
            xt2s = []
            x2_lds = []
            for k in range(NT):
                xt2 = xload.tile([P, D], F32, tag="xload", name=f"xt2_{k}")
                ld = nc.sync.dma_start(out=xt2[:],
                                       in_=x_in.ap()[k * P:(k + 1) * P, :])
                xt2s.append(xt2)
                x2_lds.append(ld)

            # ---- x pass 2 quantize + transpose ----
            qTs = []
            for k in range(NT):
                yq = scr32.tile([P, D], F32, tag="scr32", name=f"yq{k}")
                nc.scalar.activation(out=yq[:], in_=xt2s[k][:], func=AF.Identity,
                                     scale=cfull[:, k:k + 1], bias=magic_t[:])
                qn = scr16.tile([P, D], BF16, tag="qn", bufs=3, name=f"qn{k}")
                nc.vector.tensor_scalar(out=qn[:], in0=yq[:], scalar1=MAGIC,
                                        scalar2=None, op0=ALU.subtract)
                qT = qTp.tile([P, NI, P], BF16, tag="qT", name=f"qT{k}")
                nc.sync.dma_start_transpose(out=qT[:], in_=qn[:])
                qTs.append(qT)

            # ---- remaining weight blocks: after x2 on HBM ----
            gate = x2_lds[-1]
            for ob in range(1, NOB):
                wqT[ob] = wqTp.tile([P, NI, 512], BF16, tag="wqT", name=f"wqT{ob}")
                for wk in range(ob * per_ob, (ob + 1) * per_ob):
                    gate = weight_chunk(wk, gate, wqT[ob])

            # ---- matmul (ob-outer): psum[t,o] += qT_k[:,i,:].T @ wqT_ob[:,i,:] ----
            for ob in range(NOB):
                for k in range(NT):
                    ps = psump.tile([P, 512], F32, tag="ps", name=f"ps{ob}_{k}")
                    for i in range(NI):
                        nc.tensor.matmul(ps[:], qTs[k][:, i, :], wqT[ob][:, i, :],
                                         start=(i == 0), stop=(i == NI - 1))
                    osb = outp.tile([P, 512], F32, tag="osb", bufs=4,
                                    name=f"osb{ob}_{k}")
                    nc.vector.tensor_scalar(out=osb[:], in0=ps[:],
                                            scalar1=fs[:], scalar2=None,
                                            op0=ALU.mult)
                    nc.gpsimd.dma_start(
                        out=out_t.ap()[k * P:(k + 1) * P, ob * 512:(ob + 1) * 512],
                        in_=osb[:])

    nc.compile()
    return nc


_cache = {}


def _get(T, D, O, OSH):
    key = (T, D, O, OSH)
    if key not in _cache:
        _cache[key] = build(T, D, O, OSH)
    return _cache[key]


def run(x2d, weight, osh=None, trace=False):
    """x2d: [Ttot, D] f32, weight: [O, D] f32 -> [Ttot, O] f32."""
    Ttot, D = x2d.shape
    O = weight.shape[0]
    T = Ttot // N_CORES
    OSH = osh or O // N_CORES
    nc = _get(T, D, O, OSH)
    in_maps = []
    for c in range(N_CORES):
        in_maps.append({
            "x": np.ascontiguousarray(x2d[c * T:(c + 1) * T]),
            "w": weight,
            "wsh": np.ascontiguousarray(weight[c * OSH:(c + 1) * OSH]),
        })
    res = run_bass_kernel_spmd(nc, in_maps, core_ids=list(range(N_CORES)),
                               trace=trace)
    out = np.concatenate([res.results[c]["out"] for c in range(N_CORES)], axis=0)
    return out, res


def kernel(x, weight, norm_weight):
    assert np.all(norm_weight == 1.0), "general norm_weight not implemented"
    B, S, D = x.shape
    out2d, _ = run(x.reshape(B * S, D).astype(np.float32), weight.astype(np.float32))
    return out2d.reshape(B, S, weight.shape[0]).astype(np.float32)
